# revision 28
# baseline (speedup 1.0000x reference)
"""Distributed Trainium2 Bass kernel for BrosAttention.

B=2, S=1024, H=768, NH=12, DH=64:
  q,k,v = heads(hidden @ W.T + b)
  scores = q@k^T + einsum('bnid,bijd->bnij', q, bpe)   (bpe = bbox transposed)
  probs  = softmax(scores / 8)
  out    = LN(probs@v @ Wo.T + bo + hidden)

Sharding: 8 cores = 2 batches x 4 query-row blocks of 256 rows; each core
reads only its slice of bbox_pos_emb (fp8 e3m4, 16.8MB) and writes a
disjoint [256, 768] output block. No collectives.

Structure:
 - fp8 (e3m4) inputs for projections + bias einsum; weights pre-scaled x16
   on host, descaled inside the PSUM-copy activations.
 - Bias einsum: block-diag qPair weights [128,32] (2 query rows x 12 heads,
   n-major columns), 4 pairs concurrent in PE column strips; strips
   transposed back through a host-built permutation matrix so the result
   comes out n-grouped -> the scores+bias add is one contiguous
   tensor_tensor (in-place in PSUM) per [j-chunk, 6-head group].
 - kT/qT stored as 64-partition tiles: every QK matmul reads partition
   base 0 (base-64 operands + offset PSUM writes crash the HW).
 - Softmax sums folded into P@V as a 65th all-ones column of V; PV is
   interleaved with QK per j-chunk, accumulating into a persistent
   [65, 12, 128] psum tile.
"""

import os
import sys
import numpy as np

sys.path.insert(0, "/opt/trn_rl_repo")

B, S, H, NH, DH = 2, 1024, 768, 12, 64
EPS = 1e-12
P = 128
I_CORE = S * B // 8  # 256
N_CORES = 8
WSCALE = 16.0

_COMPILED = {}


def build_kernel():
    from contextlib import ExitStack
    from concourse import bacc, bass, mybir, tile

    f32 = mybir.dt.float32
    bf16 = mybir.dt.bfloat16
    f8 = mybir.dt.float8e3
    Alu = mybir.AluOpType
    Act = mybir.ActivationFunctionType
    AxisX = mybir.AxisListType.X

    SC = S // P            # 8 j chunks
    HC = H // P            # 6 hidden chunks
    IH = I_CORE // 2       # 128 i per half
    NPAIR = I_CORE // 2    # 128 i-pairs per core
    NOCT = 16              # octos (8 i's) per half
    NGRAN = 8              # bpe granule = 8 pairs (1.05 MB DMA)
    HP = NH // 2
    VH = H // 2            # 384

    nc = bacc.Bacc(None, target_bir_lowering=False, debug=False)

    d_hidT = nc.declare_dram_parameter("hidT_bf", [HC, P, S], bf16, isOutput=False)
    d_hidRT8 = nc.declare_dram_parameter("hidRT8", [HC, P, I_CORE], f8, isOutput=False)
    d_hidR = nc.declare_dram_parameter("hidR", [2, P, H], f32, isOutput=False)
    d_bpe = nc.declare_dram_parameter("bpe8", [P, NPAIR, S], f8, isOutput=False)
    d_Wq8 = nc.declare_dram_parameter("Wq8", [HC, P, H], f8, isOutput=False)
    d_Wk = nc.declare_dram_parameter("WkT_bf", [HC, P, H], bf16, isOutput=False)
    d_Wv = nc.declare_dram_parameter("WvT_bf", [HC, P, H], bf16, isOutput=False)
    d_WoT = nc.declare_dram_parameter("WoT", [HC, P, H], bf16, isOutput=False)
    d_bqcol = nc.declare_dram_parameter("bqcol", [P, HC], f32, isOutput=False)
    d_bqcol4 = nc.declare_dram_parameter("bqcol4", [P, HC], f32, isOutput=False)
    d_bkcol = nc.declare_dram_parameter("bkcol", [P, HC], f32, isOutput=False)
    d_bv = nc.declare_dram_parameter("bv_bf", [1, H], bf16, isOutput=False)
    d_bo = nc.declare_dram_parameter("bo_bf", [1, H], bf16, isOutput=False)
    d_gamma = nc.declare_dram_parameter("gamma_bf", [1, H], bf16, isOutput=False)
    d_beta = nc.declare_dram_parameter("beta_bf", [1, H], bf16, isOutput=False)
    d_perm = nc.declare_dram_parameter("perm_bf", [P, P], bf16, isOutput=False)
    d_out = nc.declare_dram_parameter("out", [2, P, H], f32, isOutput=True)

    with tile.TileContext(nc) as tc, ExitStack() as ctx:
        const_p = ctx.enter_context(tc.tile_pool(name="const", bufs=1))
        stat_p = ctx.enter_context(tc.tile_pool(name="stat", bufs=1))
        bpe_p = ctx.enter_context(tc.tile_pool(name="bpe", bufs=3))
        biasT_p = ctx.enter_context(tc.tile_pool(name="biasT", bufs=1))
        b4_p = ctx.enter_context(tc.tile_pool(name="b4", bufs=2))
        probs_p = ctx.enter_context(tc.tile_pool(name="probs", bufs=3))
        y_p = ctx.enter_context(tc.tile_pool(name="y", bufs=1))

        # ---------------- constants ----------------
        perm_bf = const_p.tile([P, P], bf16)
        nc.sync.dma_start(perm_bf[:], d_perm[:])
        ones_row = const_p.tile([1, P], bf16)
        nc.vector.memset(ones_row[:], 1.0)
        eps_t = const_p.tile([P, 1], f32)
        nc.vector.memset(eps_t[:], EPS)
        bqcol = const_p.tile([P, HC], f32)
        nc.sync.dma_start(bqcol[:], d_bqcol[:])
        bqcol4 = const_p.tile([P, HC], f32)
        nc.sync.dma_start(bqcol4[:], d_bqcol4[:])
        bkcol = const_p.tile([P, HC], f32)
        nc.sync.dma_start(bkcol[:], d_bkcol[:])
        bv_bf = const_p.tile([1, H], bf16)
        nc.sync.dma_start(bv_bf[:], d_bv[:])
        bo_bf = const_p.tile([1, H], bf16)
        nc.sync.dma_start(bo_bf[:], d_bo[:])
        gamma_r = const_p.tile([1, H], bf16)
        nc.sync.dma_start(gamma_r[:], d_gamma[:])
        beta_r = const_p.tile([1, H], bf16)
        nc.sync.dma_start(beta_r[:], d_beta[:])

        # long-lived activations (kT/qT: 64-partition tiles, base-0 reads)
        kT = stat_p.tile([DH, NH, S], bf16)
        v_sb = stat_p.tile([P, SC, NH, DH + 1], bf16)
        qT = stat_p.tile([DH, NH, I_CORE], bf16)
        qPair8 = stat_p.tile([P, NPAIR, 32], f8)    # block-diag bias weights
        nc.vector.memset(qPair8[:], 0.0)
        hidR = stat_p.tile([P, 2, H], f32)
        nc.sync.dma_start(hidR[:], d_hidR[:].transpose([1, 0, 2]))
        WoT = stat_p.tile([P, HC, H], bf16)
        nc.scalar.dma_start(WoT[:], d_WoT[:].transpose([1, 0, 2]))
        gammaB = stat_p.tile([P, H], bf16)
        betaB = stat_p.tile([P, H], bf16)
        ctxT = stat_p.tile([P, HC, IH], bf16)

        # bpe granule streaming
        bpe_tiles = {}

        def fetch_gran(g):
            t = bpe_p.tile([P, NGRAN, S], f8, name="bpeg")
            eng = nc.sync if g % 2 == 0 else nc.scalar
            eng.dma_start(t[:], d_bpe[:, g * NGRAN:(g + 1) * NGRAN, :])
            bpe_tiles[g] = t
            return t

        # ---------------- phase P: projections ----------------
        with tc.tile_pool(name="w8", bufs=1) as w8_p, \
             tc.tile_pool(name="psP", bufs=3, space=bass.MemorySpace.PSUM) \
                as psP, \
             tc.tile_pool(name="psG", bufs=2, space=bass.MemorySpace.PSUM) \
                as psG:
            Wq8 = w8_p.tile([P, HC, H], f8)
            nc.sync.dma_start(Wq8[:], d_Wq8[:].transpose([1, 0, 2]))
            hidRT8 = w8_p.tile([P, HC, I_CORE], f8)
            nc.scalar.dma_start(hidRT8[:], d_hidRT8[:].transpose([1, 0, 2]))
            WkT = w8_p.tile([P, HC, H], bf16)
            nc.scalar.dma_start(WkT[:], d_Wk[:].transpose([1, 0, 2]))
            hidT = w8_p.tile([P, HC, S], bf16)
            nc.sync.dma_start(hidT[:], d_hidT[:].transpose([1, 0, 2]))
            WvT = w8_p.tile([P, HC, H], bf16)
            nc.scalar.dma_start(WvT[:], d_Wv[:].transpose([1, 0, 2]))

            fetch_gran(0)
            fetch_gran(1)

            # Q projection (transposed): psum = 16*(Wq @ hidR^T)
            for r in range(HC):
                pq = psP.tile([P, 512], f32, name="pp")
                for kc in range(HC):
                    nc.tensor.matmul(pq[:, 0:I_CORE],
                                     Wq8[:, kc, r * P:(r + 1) * P],
                                     hidRT8[:, kc, :],
                                     start=(kc == 0), stop=(kc == HC - 1))
                for sub in range(2):
                    n = 2 * r + sub
                    srcp = pq[sub * DH:(sub + 1) * DH, 0:I_CORE]
                    bq_s = bqcol[sub * DH:(sub + 1) * DH, r:r + 1]
                    nc.vector.tensor_scalar(qT[:, n, :], srcp, 1.0 / WSCALE,
                                            bq_s, Alu.mult, Alu.add)
                    bq4_s = bqcol4[sub * DH:(sub + 1) * DH, r:r + 1]
                    nc.scalar.activation(
                        qPair8[sub * DH:(sub + 1) * DH, :, 2 * n:2 * n + 2],
                        srcp.rearrange("p (a b) -> p a b", b=2),
                        Act.Identity, scale=4.0 / WSCALE, bias=bq4_s)

            # K projection (transposed): kT = Wk @ hid^T + bk
            for r in range(HC):
                for jh in range(2):
                    pk = psP.tile([P, 512], f32, name="pp")
                    for kc in range(HC):
                        nc.tensor.matmul(pk[:], WkT[:, kc, r * P:(r + 1) * P],
                                         hidT[:, kc, jh * 512:(jh + 1) * 512],
                                         start=(kc == 0), stop=(kc == HC - 1))
                    for sub in range(2):
                        nc.vector.tensor_scalar(
                            kT[:, 2 * r + sub, jh * 512:(jh + 1) * 512],
                            pk[sub * DH:(sub + 1) * DH, :],
                            bkcol[sub * DH:(sub + 1) * DH, r:r + 1], None,
                            Alu.add)

            # V projection (natural): v = hid @ Wv^T + bv, + ones column
            for jc in range(SC):
                for vh in range(2):
                    pv = psP.tile([P, 512], f32, name="pp")
                    for kc in range(HC):
                        nc.tensor.matmul(pv[:, 0:VH],
                                         hidT[:, kc, jc * P:(jc + 1) * P],
                                         WvT[:, kc, vh * VH:(vh + 1) * VH],
                                         start=(kc == 0), stop=False)
                    nc.tensor.matmul(pv[:, 0:VH], ones_row[:],
                                     bv_bf[:, vh * VH:(vh + 1) * VH],
                                     start=False, stop=True)
                    nc.vector.tensor_copy(
                        v_sb[:, jc, vh * HP:(vh + 1) * HP, 0:DH],
                        pv[:, 0:VH].rearrange("p (a b) -> p a b", a=HP))
            nc.vector.memset(v_sb[:, :, :, DH], 1.0)

            # gamma/beta broadcast via K=1 matmuls (own pool, end of phase)
            for c in range(HC):
                pbx = psG.tile([P, P], f32, name="pbx")
                nc.tensor.matmul(pbx[:], ones_row[:],
                                 gamma_r[:, c * P:(c + 1) * P])
                nc.scalar.copy(gammaB[:, c * P:(c + 1) * P], pbx[:])
                pbx2 = psG.tile([P, P], f32, name="pbx")
                nc.tensor.matmul(pbx2[:], ones_row[:],
                                 beta_r[:, c * P:(c + 1) * P])
                nc.scalar.copy(betaB[:, c * P:(c + 1) * P], pbx2[:])

        # ---- merged-scope phases: A(h) and B(h) share one pool scope so
        # half-1's bias work overlaps half-0's serial tail on the PE ----
        with tc.tile_pool(name="psB", bufs=2, space=bass.MemorySpace.PSUM) \
                as psB, \
             tc.tile_pool(name="psT", bufs=1, space=bass.MemorySpace.PSUM) \
                as psT, \
             tc.tile_pool(name="psS", bufs=2, space=bass.MemorySpace.PSUM) \
                as psS, \
             tc.tile_pool(name="psC", bufs=1, space=bass.MemorySpace.PSUM) \
                as psC:

            def phase_A(h):
                biasT = biasT_p.tile([P, SC, NH, NOCT, 8], bf16, name="biasT")
                for g in range(8):
                    gg = h * 8 + g
                    if gg not in bpe_tiles:
                        fetch_gran(gg)
                    gt = bpe_tiles[gg]
                    if gg + 2 <= 15 and (gg + 2) not in bpe_tiles:
                        fetch_gran(gg + 2)
                    for o2 in range(2):
                        oct_ = g * 2 + o2
                        b4t = b4_p.tile([P, S], bf16, name="b4")
                        for jh in range(2):
                            pb = psB.tile([P, 512], f32, name="pb")
                            for c4 in range(4):
                                pr = h * (NPAIR // 2) + oct_ * 4 + c4
                                wi = pr % NGRAN
                                nc.tensor.matmul(
                                    pb[32 * c4:32 * c4 + 32, :],
                                    qPair8[:, pr, :],
                                    gt[:, wi, jh * 512:(jh + 1) * 512],
                                    tile_position=(0, 32 * c4))
                            nc.scalar.activation(
                                b4t[:, jh * 512:(jh + 1) * 512], pb[:],
                                Act.Copy, scale=0.25)
                        ptb = psT.tile([P, SC, P], bf16, name="ptb")
                        for jc in range(SC):
                            nc.tensor.transpose(ptb[:, jc, :],
                                                b4t[:, jc * P:(jc + 1) * P],
                                                perm_bf[:])
                        nc.vector.tensor_copy(
                            biasT[:, :, :, oct_, :],
                            ptb[:, :, 0:96].rearrange(
                                "p a (b c) -> p a b c", b=NH))
                return biasT

            def phase_Battn(h, biasT):
                pctx = psC.tile([DH + 1, NH, IH], f32, name="pctx")
                for jc in range(SC):
                    for g3 in range(3):
                        n0 = g3 * 4
                        ps_s = psS.tile([P, 4, IH], f32, name="scores")
                        for nn in range(4):
                            n = n0 + nn
                            nc.tensor.matmul(
                                ps_s[:, nn, :],
                                kT[:, n, jc * P:(jc + 1) * P],
                                qT[:, n, h * IH:(h + 1) * IH])
                        flat = ps_s[:].rearrange("p a b -> p (a b)")
                        nc.vector.tensor_tensor(
                            flat, flat,
                            biasT[:, jc, n0:n0 + 4].rearrange(
                                "p a b c -> p (a b c)"), Alu.add)
                        pt = probs_p.tile([P, 4, IH], bf16, name="probsT")
                        nc.scalar.activation(
                            pt[:].rearrange("p a b -> p (a b)"), flat,
                            Act.Exp, scale=0.125)
                        for nn in range(4):
                            n = n0 + nn
                            nc.tensor.matmul(pctx[:, n, :],
                                             v_sb[:, jc, n, :],
                                             pt[:, nn, :],
                                             start=(jc == 0),
                                             stop=(jc == SC - 1),
                                             skip_group_check=True)
                return pctx

            def phase_tail(h, pctx):
                sumsB = y_p.tile([1, NH, IH], bf16, name="sumsB")
                nc.vector.tensor_copy(sumsB[0:1, :, :], pctx[DH:DH + 1, :, :])
                for g3 in range(3):
                    prs = psS.tile([DH, 4, IH], f32, name="scores")
                    for nn in range(4):
                        nc.tensor.matmul(prs[:, nn, :], ones_row[:, 0:DH],
                                         sumsB[0:1, g3 * 4 + nn, :])
                    lnS = y_p.tile([DH, 4, IH], f32, name="lnS")
                    nc.scalar.activation(
                        lnS[:].rearrange("p a b -> p (a b)"),
                        prs[:].rearrange("p a b -> p (a b)"), Act.Ln)
                    precS = y_p.tile([DH, 4, IH], f32, name="precS")
                    nc.scalar.activation(
                        precS[:].rearrange("p a b -> p (a b)"),
                        lnS[:].rearrange("p a b -> p (a b)"), Act.Exp,
                        scale=-1.0)
                    for sub in range(2):
                        nc.vector.tensor_tensor(
                            ctxT[sub * DH:(sub + 1) * DH,
                                 2 * g3:2 * g3 + 2, :],
                            pctx[0:DH, g3 * 4 + sub:g3 * 4 + 4:2, :],
                            precS[:, sub:4:2, :], Alu.mult)

                # ---- O proj + residual + LN ----
                y = y_p.tile([P, H], f32, name="yy")
                for vh in range(2):
                    py = psS.tile([P, VH], f32, name="scores")
                    for kc in range(HC):
                        nc.tensor.matmul(py[:], ctxT[:, kc, :],
                                         WoT[:, kc, vh * VH:(vh + 1) * VH],
                                         start=(kc == 0), stop=False)
                    nc.tensor.matmul(py[:], ones_row[:],
                                     bo_bf[:, vh * VH:(vh + 1) * VH],
                                     start=False, stop=True)
                    nc.vector.tensor_tensor(y[:, vh * VH:(vh + 1) * VH],
                                            py[:],
                                            hidR[:, h, vh * VH:(vh + 1) * VH],
                                            Alu.add)
                mu = y_p.tile([P, 1], f32, name="mu")
                nc.vector.tensor_reduce(mu[:], y[:], AxisX, Alu.add)
                nc.vector.tensor_scalar(mu[:], mu[:], 1.0 / H, None, Alu.mult)
                yc = y_p.tile([P, H], f32, name="yc")
                nc.vector.tensor_scalar(yc[:], y[:], mu[:], None, Alu.subtract)
                ssq = y_p.tile([P, 1], f32, name="ssq")
                nc.scalar.activation(y[:], yc[:], Act.Square, accum_out=ssq[:])
                std = y_p.tile([P, 1], f32, name="std")
                nc.scalar.activation(std[:], ssq[:], Act.Sqrt,
                                     scale=1.0 / H, bias=eps_t[:])
                rstd = y_p.tile([P, 1], f32, name="rstd")
                nc.vector.reciprocal(rstd[:], std[:])
                o1 = y_p.tile([P, H], f32, name="o1")
                nc.vector.tensor_scalar(o1[:], yc[:], rstd[:], None, Alu.mult)
                nc.vector.tensor_tensor(o1[:], o1[:], gammaB[:], Alu.mult)
                nc.vector.tensor_tensor(o1[:], o1[:], betaB[:], Alu.add)
                nc.sync.dma_start(d_out[h], o1[:])

            bT0 = phase_A(0)
            c0 = phase_Battn(0, bT0)
            bT1 = phase_A(1)       # overlaps half-0 tail on the PE
            phase_tail(0, c0)
            c1 = phase_Battn(1, bT1)
            phase_tail(1, c1)

    nc.compile()
    return nc


def _build_perm():
    """Permutation: transpose output column f <- b4 strip row sigma(f).
    f-order: (n, c4, par) for f<96; sigma(f) = 32*c4 + 2*n + par."""
    perm = np.zeros((P, P), np.float32)
    for n in range(NH):
        for c4 in range(4):
            for par in range(2):
                f = n * 8 + c4 * 2 + par
                perm[32 * c4 + 2 * n + par, f] = 1.0
    for c4 in range(4):
        for u in range(8):
            perm[32 * c4 + 24 + u, 96 + c4 * 8 + u] = 1.0
    return perm


def _shard_inputs(inputs):
    import ml_dtypes
    bf = ml_dtypes.bfloat16
    f8 = ml_dtypes.float8_e3m4
    hs = np.ascontiguousarray(np.asarray(inputs["hidden_states"]), np.float32)
    bpe = np.asarray(inputs["bbox_pos_emb"])
    HC = H // P

    perm = _build_perm().astype(bf)
    Wq8 = np.ascontiguousarray(
        (np.asarray(inputs["Wq"], np.float32).T * WSCALE).astype(f8)).reshape(
            HC, P, H)
    WkT = np.ascontiguousarray(
        np.asarray(inputs["Wk"], np.float32).T.astype(bf)).reshape(HC, P, H)
    WvT = np.ascontiguousarray(
        np.asarray(inputs["Wv"], np.float32).T.astype(bf)).reshape(HC, P, H)
    WoT = np.ascontiguousarray(
        np.asarray(inputs["Wo"], np.float32).T.astype(bf)).reshape(HC, P, H)

    def col(name, scale=1.0):
        v = np.asarray(inputs[name], np.float32) * scale
        return np.ascontiguousarray(v.reshape(HC, P).T)  # [P, HC]

    bqcol = col("bq")
    bqcol4 = col("bq", 4.0)
    bkcol = col("bk")
    bv_bf = np.asarray(inputs["bv"], np.float32).astype(bf).reshape(1, H)
    bo_bf = np.asarray(inputs["bo"], np.float32).astype(bf).reshape(1, H)
    gamma_bf = np.asarray(inputs["ln_gamma"], np.float32).astype(bf).reshape(1, H)
    beta_bf = np.asarray(inputs["ln_beta"], np.float32).astype(bf).reshape(1, H)

    hsT = {b: np.ascontiguousarray(hs[b].T).astype(bf).reshape(HC, P, S)
           for b in range(B)}

    in_maps = []
    for c in range(N_CORES):
        b = c // 4
        q0 = (c % 4) * I_CORE
        # bpe8 [128, 128, 1024]: row p=(i%2)*64+d, [pair, j]
        x = bpe[q0:q0 + I_CORE, :, b, :]            # [256 i, 1024 j, 64 d]
        x = np.asarray(x, np.float32).reshape(I_CORE // 2, 2, S, DH)
        x = x.transpose(1, 3, 0, 2).reshape(P, I_CORE // 2, S)
        m = {
            "hidT_bf": hsT[b],
            "hidRT8": np.ascontiguousarray(
                hs[b, q0:q0 + I_CORE].T).astype(f8).reshape(HC, P, I_CORE),
            "hidR": np.ascontiguousarray(
                hs[b, q0:q0 + I_CORE].reshape(2, P, H)),
            "bpe8": np.ascontiguousarray(x).astype(f8),
            "WoT": WoT, "Wq8": Wq8, "WkT_bf": WkT, "WvT_bf": WvT,
            "bqcol": bqcol, "bqcol4": bqcol4, "bkcol": bkcol,
            "bv_bf": bv_bf, "bo_bf": bo_bf,
            "gamma_bf": gamma_bf, "beta_bf": beta_bf,
            "perm_bf": perm,
        }
        in_maps.append(m)
    return in_maps


def _install_ntff_shim():
    """The agent image's antenv lacks axon_hooks; recreate the NTFF profile
    hook via ctypes against libaxon_pjrt.so so trace=True yields
    exec_time_ns + a perfetto trace."""
    import sys as _sys
    if "antenv.axon_hooks" in _sys.modules:
        return
    import types, ctypes, contextlib
    so_path = "/opt/axon/libaxon_pjrt.so"
    mod = types.ModuleType("antenv.axon_hooks")
    _state = {}

    def get_axon_ntff_profile_hook():
        if "hook" in _state:
            return _state["hook"]
        try:
            lib = ctypes.CDLL(so_path)
            if not hasattr(lib, "axon_start_nrt_profile"):
                _state["hook"] = None
                return None
            lib.axon_start_nrt_profile.argtypes = [
                ctypes.POINTER(ctypes.c_int64), ctypes.c_size_t]
            lib.axon_start_nrt_profile.restype = ctypes.c_int64
            lib.axon_stop_nrt_profile.argtypes = [ctypes.c_char_p]
            lib.axon_stop_nrt_profile.restype = ctypes.c_int64
        except OSError:
            _state["hook"] = None
            return None

        @contextlib.contextmanager
        def _hook(output_dir, device_ids):
            import jax
            jax.devices()
            if device_ids:
                ids = (ctypes.c_int64 * len(device_ids))(*device_ids)
                rc = lib.axon_start_nrt_profile(ids, len(device_ids))
            else:
                rc = lib.axon_start_nrt_profile(None, 0)
            if rc != 0:
                raise RuntimeError(f"axon_start_nrt_profile rc={rc}")
            try:
                yield
            finally:
                n = lib.axon_stop_nrt_profile(str(output_dir).encode())
                print(f"ntff profile: {n} file(s) written to {output_dir}")

        _state["hook"] = _hook
        return _hook

    mod.get_axon_ntff_profile_hook = get_axon_ntff_profile_hook
    _sys.modules["antenv.axon_hooks"] = mod


def kernel(**inputs):
    from concourse.bass_utils import run_bass_kernel_spmd

    if os.environ.get("BASS_KERNEL_TRACE"):
        _install_ntff_shim()
        import concourse.bass_utils as _bu
        _bu.upload_artifacts = lambda tmpdir: f"file://{tmpdir}"

    if "nc" not in _COMPILED:
        _COMPILED["nc"] = build_kernel()
    nc = _COMPILED["nc"]
    in_maps = _shard_inputs(inputs)
    res = run_bass_kernel_spmd(nc, in_maps, core_ids=list(range(N_CORES)),
                               trace=bool(os.environ.get("BASS_KERNEL_TRACE")))
    _COMPILED["last_result"] = res
    out = np.zeros((B, S, H), dtype=np.float32)
    for c in range(N_CORES):
        b = c // 4
        q0 = (c % 4) * I_CORE
        out[b, q0:q0 + I_CORE] = np.asarray(
            res.results[c]["out"]).reshape(I_CORE, H)
    return out


# revision 32
# speedup vs baseline: 1.0704x; 1.0704x over previous
"""Distributed Trainium2 Bass kernel for BrosAttention.

B=2, S=1024, H=768, NH=12, DH=64:
  q,k,v = heads(hidden @ W.T + b)
  scores = q@k^T + einsum('bnid,bijd->bnij', q, bpe)   (bpe = bbox transposed)
  probs  = softmax(scores / 8)
  out    = LN(probs@v @ Wo.T + bo + hidden)

Sharding: 8 cores = 2 batches x 4 query-row blocks of 256 rows; each core
reads only its slice of bbox_pos_emb (fp8 e3m4, 16.8MB) and writes a
disjoint [256, 768] output block. No collectives.

Structure:
 - fp8 (e3m4) inputs for projections + bias einsum; weights pre-scaled x16
   on host, descaled inside the PSUM-copy activations.
 - Bias einsum: block-diag qPair weights [128,32] (2 query rows x 12 heads,
   n-major columns), 4 pairs concurrent in PE column strips; strips
   transposed back through a host-built permutation matrix so the result
   comes out n-grouped -> the scores+bias add is one contiguous
   tensor_tensor (in-place in PSUM) per [j-chunk, 6-head group].
 - kT/qT stored as 64-partition tiles: every QK matmul reads partition
   base 0 (base-64 operands + offset PSUM writes crash the HW).
 - Softmax sums folded into P@V as a 65th all-ones column of V; PV is
   interleaved with QK per j-chunk, accumulating into a persistent
   [65, 12, 128] psum tile.
"""

import os
import sys
import numpy as np

sys.path.insert(0, "/opt/trn_rl_repo")

B, S, H, NH, DH = 2, 1024, 768, 12, 64
EPS = 1e-12
P = 128
I_CORE = S * B // 8  # 256
N_CORES = 8
WSCALE = 16.0

_COMPILED = {}


def build_kernel():
    from contextlib import ExitStack
    from concourse import bacc, bass, mybir, tile

    f32 = mybir.dt.float32
    bf16 = mybir.dt.bfloat16
    f8 = mybir.dt.float8e3
    Alu = mybir.AluOpType
    Act = mybir.ActivationFunctionType
    AxisX = mybir.AxisListType.X

    SC = S // P            # 8 j chunks
    HC = H // P            # 6 hidden chunks
    IH = I_CORE // 2       # 128 i per half
    NPAIR = I_CORE // 2    # 128 i-pairs per core
    NOCT = 16              # octos (8 i's) per half
    NGRAN = 8              # bpe granule = 8 pairs (1.05 MB DMA)
    HP = NH // 2
    VH = H // 2            # 384

    nc = bacc.Bacc(None, target_bir_lowering=False, debug=False)

    d_hidT = nc.declare_dram_parameter("hidT_bf", [HC, P, S], bf16, isOutput=False)
    d_hidRT8 = nc.declare_dram_parameter("hidRT8", [HC, P, I_CORE], f8, isOutput=False)
    d_hidR = nc.declare_dram_parameter("hidR", [2, P, H], f32, isOutput=False)
    d_bpe = nc.declare_dram_parameter("bpe8", [P, NPAIR, S], f8, isOutput=False)
    d_Wq8 = nc.declare_dram_parameter("Wq8", [HC, P, H], f8, isOutput=False)
    d_Wk = nc.declare_dram_parameter("WkT_bf", [HC, P, H], bf16, isOutput=False)
    d_Wv = nc.declare_dram_parameter("WvT_bf", [HC, P, H], bf16, isOutput=False)
    d_WoT = nc.declare_dram_parameter("WoT", [HC, P, H], bf16, isOutput=False)
    d_bqcol = nc.declare_dram_parameter("bqcol", [P, HC], f32, isOutput=False)
    d_bqcol4 = nc.declare_dram_parameter("bqcol4", [P, HC], f32, isOutput=False)
    d_bkcol = nc.declare_dram_parameter("bkcol", [P, HC], f32, isOutput=False)
    d_bv = nc.declare_dram_parameter("bv_bf", [1, H], bf16, isOutput=False)
    d_bo = nc.declare_dram_parameter("bo_bf", [1, H], bf16, isOutput=False)
    d_gamma = nc.declare_dram_parameter("gamma_bf", [1, H], bf16, isOutput=False)
    d_beta = nc.declare_dram_parameter("beta_bf", [1, H], bf16, isOutput=False)
    d_perm = nc.declare_dram_parameter("perm_bf", [P, P], bf16, isOutput=False)
    d_out = nc.declare_dram_parameter("out", [2, P, H], f32, isOutput=True)

    with tile.TileContext(nc) as tc, ExitStack() as ctx:
        const_p = ctx.enter_context(tc.tile_pool(name="const", bufs=1))
        stat_p = ctx.enter_context(tc.tile_pool(name="stat", bufs=1))
        bpe_p = ctx.enter_context(tc.tile_pool(name="bpe", bufs=4))
        biasT_p = ctx.enter_context(tc.tile_pool(name="biasT", bufs=1))
        b4_p = ctx.enter_context(tc.tile_pool(name="b4", bufs=2))
        probs_p = ctx.enter_context(tc.tile_pool(name="probs", bufs=3))
        sE_p = ctx.enter_context(tc.tile_pool(name="sE", bufs=2))
        y_p = ctx.enter_context(tc.tile_pool(name="y", bufs=1))

        # ---------------- constants ----------------
        perm_bf = const_p.tile([P, P], bf16)
        nc.sync.dma_start(perm_bf[:], d_perm[:])
        ones_row = const_p.tile([1, P], bf16)
        nc.vector.memset(ones_row[:], 1.0)
        eps_t = const_p.tile([P, 1], f32)
        nc.vector.memset(eps_t[:], EPS)
        bqcol = const_p.tile([P, HC], f32)
        nc.sync.dma_start(bqcol[:], d_bqcol[:])
        bqcol4 = const_p.tile([P, HC], f32)
        nc.sync.dma_start(bqcol4[:], d_bqcol4[:])
        bkcol = const_p.tile([P, HC], f32)
        nc.sync.dma_start(bkcol[:], d_bkcol[:])
        bv_bf = const_p.tile([1, H], bf16)
        nc.sync.dma_start(bv_bf[:], d_bv[:])
        bo_bf = const_p.tile([1, H], bf16)
        nc.sync.dma_start(bo_bf[:], d_bo[:])
        gamma_r = const_p.tile([1, H], bf16)
        nc.sync.dma_start(gamma_r[:], d_gamma[:])
        beta_r = const_p.tile([1, H], bf16)
        nc.sync.dma_start(beta_r[:], d_beta[:])

        # long-lived activations (kT/qT: 64-partition tiles, base-0 reads)
        kT = stat_p.tile([DH, NH, S], bf16)
        v_sb = stat_p.tile([P, SC, NH, DH + 1], bf16)
        qT = stat_p.tile([DH, NH, I_CORE], bf16)
        qPair8 = stat_p.tile([P, NPAIR, 32], f8)    # block-diag bias weights
        nc.vector.memset(qPair8[:], 0.0)
        hidR = stat_p.tile([P, 2, H], f32)
        nc.sync.dma_start(hidR[:], d_hidR[:].transpose([1, 0, 2]))
        WoT = stat_p.tile([P, HC, H], bf16)
        nc.scalar.dma_start(WoT[:], d_WoT[:].transpose([1, 0, 2]))
        gammaB = stat_p.tile([P, H], bf16)
        betaB = stat_p.tile([P, H], bf16)
        ctxT = stat_p.tile([P, HC, IH], bf16)

        # bpe granule streaming
        bpe_tiles = {}

        def fetch_gran(g):
            t = bpe_p.tile([P, NGRAN, S], f8, name="bpeg")
            eng = nc.sync if g % 2 == 0 else nc.scalar
            eng.dma_start(t[:], d_bpe[:, g * NGRAN:(g + 1) * NGRAN, :])
            bpe_tiles[g] = t
            return t

        # ---------------- phase P: projections ----------------
        with tc.tile_pool(name="w8", bufs=1) as w8_p, \
             tc.tile_pool(name="psP", bufs=3, space=bass.MemorySpace.PSUM) \
                as psP, \
             tc.tile_pool(name="psG", bufs=2, space=bass.MemorySpace.PSUM) \
                as psG:
            Wq8 = w8_p.tile([P, HC, H], f8)
            nc.sync.dma_start(Wq8[:], d_Wq8[:].transpose([1, 0, 2]))
            hidRT8 = w8_p.tile([P, HC, I_CORE], f8)
            nc.scalar.dma_start(hidRT8[:], d_hidRT8[:].transpose([1, 0, 2]))
            WkT = w8_p.tile([P, HC, H], bf16)
            nc.scalar.dma_start(WkT[:], d_Wk[:].transpose([1, 0, 2]))
            hidT = w8_p.tile([P, HC, S], bf16)
            nc.sync.dma_start(hidT[:], d_hidT[:].transpose([1, 0, 2]))
            WvT = w8_p.tile([P, HC, H], bf16)
            nc.scalar.dma_start(WvT[:], d_Wv[:].transpose([1, 0, 2]))

            fetch_gran(0)
            fetch_gran(1)

            # Q projection (transposed): psum = 16*(Wq @ hidR^T)
            for r in range(HC):
                pq = psP.tile([P, 512], f32, name="pp")
                for kc in range(HC):
                    nc.tensor.matmul(pq[:, 0:I_CORE],
                                     Wq8[:, kc, r * P:(r + 1) * P],
                                     hidRT8[:, kc, :],
                                     start=(kc == 0), stop=(kc == HC - 1))
                for sub in range(2):
                    n = 2 * r + sub
                    srcp = pq[sub * DH:(sub + 1) * DH, 0:I_CORE]
                    bq_s = bqcol[sub * DH:(sub + 1) * DH, r:r + 1]
                    nc.vector.tensor_scalar(qT[:, n, :], srcp, 1.0 / WSCALE,
                                            bq_s, Alu.mult, Alu.add)
                    bq4_s = bqcol4[sub * DH:(sub + 1) * DH, r:r + 1]
                    nc.scalar.activation(
                        qPair8[sub * DH:(sub + 1) * DH, :, 2 * n:2 * n + 2],
                        srcp.rearrange("p (a b) -> p a b", b=2),
                        Act.Identity, scale=4.0 / WSCALE, bias=bq4_s)

            # K projection (transposed): kT = Wk @ hid^T + bk
            for r in range(HC):
                for jh in range(2):
                    pk = psP.tile([P, 512], f32, name="pp")
                    for kc in range(HC):
                        nc.tensor.matmul(pk[:], WkT[:, kc, r * P:(r + 1) * P],
                                         hidT[:, kc, jh * 512:(jh + 1) * 512],
                                         start=(kc == 0), stop=(kc == HC - 1))
                    for sub in range(2):
                        nc.vector.tensor_scalar(
                            kT[:, 2 * r + sub, jh * 512:(jh + 1) * 512],
                            pk[sub * DH:(sub + 1) * DH, :],
                            bkcol[sub * DH:(sub + 1) * DH, r:r + 1], None,
                            Alu.add)

            # V projection (natural): v = hid @ Wv^T + bv, + ones column
            for jc in range(SC):
                for vh in range(2):
                    pv = psP.tile([P, 512], f32, name="pp")
                    for kc in range(HC):
                        nc.tensor.matmul(pv[:, 0:VH],
                                         hidT[:, kc, jc * P:(jc + 1) * P],
                                         WvT[:, kc, vh * VH:(vh + 1) * VH],
                                         start=(kc == 0), stop=False)
                    nc.tensor.matmul(pv[:, 0:VH], ones_row[:],
                                     bv_bf[:, vh * VH:(vh + 1) * VH],
                                     start=False, stop=True)
                    nc.vector.tensor_copy(
                        v_sb[:, jc, vh * HP:(vh + 1) * HP, 0:DH],
                        pv[:, 0:VH].rearrange("p (a b) -> p a b", a=HP))
            nc.vector.memset(v_sb[:, :, :, DH], 1.0)

            # gamma/beta broadcast via K=1 matmuls (own pool, end of phase)
            for c in range(HC):
                pbx = psG.tile([P, P], f32, name="pbx")
                nc.tensor.matmul(pbx[:], ones_row[:],
                                 gamma_r[:, c * P:(c + 1) * P])
                nc.scalar.copy(gammaB[:, c * P:(c + 1) * P], pbx[:])
                pbx2 = psG.tile([P, P], f32, name="pbx")
                nc.tensor.matmul(pbx2[:], ones_row[:],
                                 beta_r[:, c * P:(c + 1) * P])
                nc.scalar.copy(betaB[:, c * P:(c + 1) * P], pbx2[:])

        # ---- merged-scope phases: A(h) and B(h) share one pool scope so
        # half-1's bias work overlaps half-0's serial tail on the PE ----
        with tc.tile_pool(name="psB", bufs=2, space=bass.MemorySpace.PSUM) \
                as psB, \
             tc.tile_pool(name="psT", bufs=1, space=bass.MemorySpace.PSUM) \
                as psT, \
             tc.tile_pool(name="psS", bufs=2, space=bass.MemorySpace.PSUM) \
                as psS, \
             tc.tile_pool(name="psC", bufs=1, space=bass.MemorySpace.PSUM) \
                as psC:

            def phase_A(h, g_lo=0, g_hi=8, biasT=None):
                if biasT is None:
                    biasT = biasT_p.tile([P, SC, NH, NOCT, 8], bf16,
                                         name="biasT")
                for g in range(g_lo, g_hi):
                    gg = h * 8 + g
                    if gg not in bpe_tiles:
                        fetch_gran(gg)
                    gt = bpe_tiles[gg]
                    if gg + 3 <= 15 and (gg + 3) not in bpe_tiles:
                        fetch_gran(gg + 3)
                    for o2 in range(2):
                        oct_ = g * 2 + o2
                        b4t = b4_p.tile([P, S], bf16, name="b4")
                        for jh in range(2):
                            pb = psB.tile([P, 512], f32, name="pb")
                            for c4 in range(4):
                                pr = h * (NPAIR // 2) + oct_ * 4 + c4
                                wi = pr % NGRAN
                                nc.tensor.matmul(
                                    pb[32 * c4:32 * c4 + 32, :],
                                    qPair8[:, pr, :],
                                    gt[:, wi, jh * 512:(jh + 1) * 512],
                                    tile_position=(0, 32 * c4))
                            nc.scalar.activation(
                                b4t[:, jh * 512:(jh + 1) * 512], pb[:],
                                Act.Copy, scale=0.25)
                        ptb = psT.tile([P, SC, P], bf16, name="ptb")
                        for jc in range(SC):
                            nc.tensor.transpose(ptb[:, jc, :],
                                                b4t[:, jc * P:(jc + 1) * P],
                                                perm_bf[:])
                        nc.vector.tensor_copy(
                            biasT[:, :, :, oct_, :],
                            ptb[:, :, 0:96].rearrange(
                                "p a (b c) -> p a b c", b=NH))
                return biasT

            def phase_Battn(h, biasT):
                pctx = psC.tile([DH + 1, NH, IH], f32, name="pctx")
                for jc in range(SC):
                    for g3 in range(3):
                        n0 = g3 * 4
                        ps_s = psS.tile([P, 4, IH], f32, name="scores")
                        for nn in range(4):
                            n = n0 + nn
                            nc.tensor.matmul(
                                ps_s[:, nn, :],
                                kT[:, n, jc * P:(jc + 1) * P],
                                qT[:, n, h * IH:(h + 1) * IH])
                        if g3 != 1:
                            sE = sE_p.tile([P, 8, IH], f32, name="sE")
                        slot = 1 if g3 == 1 else 0
                        nc.vector.tensor_tensor(
                            sE[:, slot * 4:slot * 4 + 4, :].rearrange(
                                "p a b -> p (a b)"),
                            ps_s[:].rearrange("p a b -> p (a b)"),
                            biasT[:, jc, n0:n0 + 4].rearrange(
                                "p a b c -> p (a b c)"), Alu.add)
                        if g3 == 1:
                            pt = probs_p.tile([P, 8, IH], bf16, name="probsT")
                            nc.scalar.activation(
                                pt[:].rearrange("p a b -> p (a b)"),
                                sE[:].rearrange("p a b -> p (a b)"),
                                Act.Exp, scale=0.125)
                            for nn in range(8):
                                nc.tensor.matmul(pctx[:, nn, :],
                                                 v_sb[:, jc, nn, :],
                                                 pt[:, nn, :],
                                                 start=(jc == 0),
                                                 stop=(jc == SC - 1),
                                                 skip_group_check=True)
                        elif g3 == 2:
                            pt2 = probs_p.tile([P, 4, IH], bf16, name="probs2")
                            nc.scalar.activation(
                                pt2[:].rearrange("p a b -> p (a b)"),
                                sE[:, 0:4, :].rearrange("p a b -> p (a b)"),
                                Act.Exp, scale=0.125)
                            for nn in range(4):
                                n = 8 + nn
                                nc.tensor.matmul(pctx[:, n, :],
                                                 v_sb[:, jc, n, :],
                                                 pt2[:, nn, :],
                                                 start=(jc == 0),
                                                 stop=(jc == SC - 1),
                                                 skip_group_check=True)
                return pctx

            def phase_tail(h, pctx):
                sumsB = y_p.tile([1, NH, IH], bf16, name="sumsB")
                nc.vector.tensor_copy(sumsB[0:1, :, :], pctx[DH:DH + 1, :, :])
                for g3 in range(3):
                    prs = psS.tile([DH, 4, IH], f32, name="scores")
                    for nn in range(4):
                        nc.tensor.matmul(prs[:, nn, :], ones_row[:, 0:DH],
                                         sumsB[0:1, g3 * 4 + nn, :])
                    precS = y_p.tile([DH, 4, IH], f32, name="precS")
                    nc.vector.reciprocal_approx_fast(
                        precS[:].rearrange("p a b -> p (a b)"),
                        prs[:].rearrange("p a b -> p (a b)"))
                    for sub in range(2):
                        nc.vector.tensor_tensor(
                            ctxT[sub * DH:(sub + 1) * DH,
                                 2 * g3:2 * g3 + 2, :],
                            pctx[0:DH, g3 * 4 + sub:g3 * 4 + 4:2, :],
                            precS[:, sub:4:2, :], Alu.mult)

                # ---- O proj + residual + LN ----
                y = y_p.tile([P, H], f32, name="yy")
                for vh in range(2):
                    py = psS.tile([P, VH], f32, name="scores")
                    for kc in range(HC):
                        nc.tensor.matmul(py[:], ctxT[:, kc, :],
                                         WoT[:, kc, vh * VH:(vh + 1) * VH],
                                         start=(kc == 0), stop=False)
                    nc.tensor.matmul(py[:], ones_row[:],
                                     bo_bf[:, vh * VH:(vh + 1) * VH],
                                     start=False, stop=True)
                    nc.vector.tensor_tensor(y[:, vh * VH:(vh + 1) * VH],
                                            py[:],
                                            hidR[:, h, vh * VH:(vh + 1) * VH],
                                            Alu.add)
                mu = y_p.tile([P, 1], f32, name="mu")
                nc.vector.tensor_reduce(mu[:], y[:], AxisX, Alu.add)
                nc.vector.tensor_scalar(mu[:], mu[:], 1.0 / H, None, Alu.mult)
                yc = y_p.tile([P, H], f32, name="yc")
                nc.vector.tensor_scalar(yc[:], y[:], mu[:], None, Alu.subtract)
                ssq = y_p.tile([P, 1], f32, name="ssq")
                nc.scalar.activation(y[:], yc[:], Act.Square, accum_out=ssq[:])
                std = y_p.tile([P, 1], f32, name="std")
                nc.scalar.activation(std[:], ssq[:], Act.Sqrt,
                                     scale=1.0 / H, bias=eps_t[:])
                rstd = y_p.tile([P, 1], f32, name="rstd")
                nc.vector.reciprocal(rstd[:], std[:])
                o1 = y_p.tile([P, H], f32, name="o1")
                nc.vector.tensor_scalar(o1[:], yc[:], rstd[:], None, Alu.mult)
                nc.vector.tensor_tensor(o1[:], o1[:], gammaB[:], Alu.mult)
                nc.vector.tensor_tensor(o1[:], o1[:], betaB[:], Alu.add)
                nc.sync.dma_start(d_out[h], o1[:])

            bT0 = phase_A(0)
            c0 = phase_Battn(0, bT0)
            bT1 = phase_A(1, 0, 5)        # overlaps half-0 tail on the PE
            phase_tail(0, c0)
            phase_A(1, 5, 8, biasT=bT1)   # dense PE burst re-warms HAM
            c1 = phase_Battn(1, bT1)
            phase_tail(1, c1)

    nc.compile()
    return nc


def _build_perm():
    """Permutation: transpose output column f <- b4 strip row sigma(f).
    f-order: (n, c4, par) for f<96; sigma(f) = 32*c4 + 2*n + par."""
    perm = np.zeros((P, P), np.float32)
    for n in range(NH):
        for c4 in range(4):
            for par in range(2):
                f = n * 8 + c4 * 2 + par
                perm[32 * c4 + 2 * n + par, f] = 1.0
    for c4 in range(4):
        for u in range(8):
            perm[32 * c4 + 24 + u, 96 + c4 * 8 + u] = 1.0
    return perm


def _shard_inputs(inputs):
    import ml_dtypes
    bf = ml_dtypes.bfloat16
    f8 = ml_dtypes.float8_e3m4
    hs = np.ascontiguousarray(np.asarray(inputs["hidden_states"]), np.float32)
    bpe = np.asarray(inputs["bbox_pos_emb"])
    HC = H // P

    perm = _build_perm().astype(bf)
    Wq8 = np.ascontiguousarray(
        (np.asarray(inputs["Wq"], np.float32).T * WSCALE).astype(f8)).reshape(
            HC, P, H)
    WkT = np.ascontiguousarray(
        np.asarray(inputs["Wk"], np.float32).T.astype(bf)).reshape(HC, P, H)
    WvT = np.ascontiguousarray(
        np.asarray(inputs["Wv"], np.float32).T.astype(bf)).reshape(HC, P, H)
    WoT = np.ascontiguousarray(
        np.asarray(inputs["Wo"], np.float32).T.astype(bf)).reshape(HC, P, H)

    def col(name, scale=1.0):
        v = np.asarray(inputs[name], np.float32) * scale
        return np.ascontiguousarray(v.reshape(HC, P).T)  # [P, HC]

    bqcol = col("bq")
    bqcol4 = col("bq", 4.0)
    bkcol = col("bk")
    bv_bf = np.asarray(inputs["bv"], np.float32).astype(bf).reshape(1, H)
    bo_bf = np.asarray(inputs["bo"], np.float32).astype(bf).reshape(1, H)
    gamma_bf = np.asarray(inputs["ln_gamma"], np.float32).astype(bf).reshape(1, H)
    beta_bf = np.asarray(inputs["ln_beta"], np.float32).astype(bf).reshape(1, H)

    hsT = {b: np.ascontiguousarray(hs[b].T).astype(bf).reshape(HC, P, S)
           for b in range(B)}

    in_maps = []
    for c in range(N_CORES):
        b = c // 4
        q0 = (c % 4) * I_CORE
        # bpe8 [128, 128, 1024]: row p=(i%2)*64+d, [pair, j]
        x = bpe[q0:q0 + I_CORE, :, b, :]            # [256 i, 1024 j, 64 d]
        x = np.asarray(x, np.float32).reshape(I_CORE // 2, 2, S, DH)
        x = x.transpose(1, 3, 0, 2).reshape(P, I_CORE // 2, S)
        m = {
            "hidT_bf": hsT[b],
            "hidRT8": np.ascontiguousarray(
                hs[b, q0:q0 + I_CORE].T).astype(f8).reshape(HC, P, I_CORE),
            "hidR": np.ascontiguousarray(
                hs[b, q0:q0 + I_CORE].reshape(2, P, H)),
            "bpe8": np.ascontiguousarray(x).astype(f8),
            "WoT": WoT, "Wq8": Wq8, "WkT_bf": WkT, "WvT_bf": WvT,
            "bqcol": bqcol, "bqcol4": bqcol4, "bkcol": bkcol,
            "bv_bf": bv_bf, "bo_bf": bo_bf,
            "gamma_bf": gamma_bf, "beta_bf": beta_bf,
            "perm_bf": perm,
        }
        in_maps.append(m)
    return in_maps


def _install_ntff_shim():
    """The agent image's antenv lacks axon_hooks; recreate the NTFF profile
    hook via ctypes against libaxon_pjrt.so so trace=True yields
    exec_time_ns + a perfetto trace."""
    import sys as _sys
    if "antenv.axon_hooks" in _sys.modules:
        return
    import types, ctypes, contextlib
    so_path = "/opt/axon/libaxon_pjrt.so"
    mod = types.ModuleType("antenv.axon_hooks")
    _state = {}

    def get_axon_ntff_profile_hook():
        if "hook" in _state:
            return _state["hook"]
        try:
            lib = ctypes.CDLL(so_path)
            if not hasattr(lib, "axon_start_nrt_profile"):
                _state["hook"] = None
                return None
            lib.axon_start_nrt_profile.argtypes = [
                ctypes.POINTER(ctypes.c_int64), ctypes.c_size_t]
            lib.axon_start_nrt_profile.restype = ctypes.c_int64
            lib.axon_stop_nrt_profile.argtypes = [ctypes.c_char_p]
            lib.axon_stop_nrt_profile.restype = ctypes.c_int64
        except OSError:
            _state["hook"] = None
            return None

        @contextlib.contextmanager
        def _hook(output_dir, device_ids):
            import jax
            jax.devices()
            if device_ids:
                ids = (ctypes.c_int64 * len(device_ids))(*device_ids)
                rc = lib.axon_start_nrt_profile(ids, len(device_ids))
            else:
                rc = lib.axon_start_nrt_profile(None, 0)
            if rc != 0:
                raise RuntimeError(f"axon_start_nrt_profile rc={rc}")
            try:
                yield
            finally:
                n = lib.axon_stop_nrt_profile(str(output_dir).encode())
                print(f"ntff profile: {n} file(s) written to {output_dir}")

        _state["hook"] = _hook
        return _hook

    mod.get_axon_ntff_profile_hook = get_axon_ntff_profile_hook
    _sys.modules["antenv.axon_hooks"] = mod


def kernel(**inputs):
    from concourse.bass_utils import run_bass_kernel_spmd

    if os.environ.get("BASS_KERNEL_TRACE"):
        _install_ntff_shim()
        import concourse.bass_utils as _bu
        _bu.upload_artifacts = lambda tmpdir: f"file://{tmpdir}"

    if "nc" not in _COMPILED:
        _COMPILED["nc"] = build_kernel()
    nc = _COMPILED["nc"]
    in_maps = _shard_inputs(inputs)
    res = run_bass_kernel_spmd(nc, in_maps, core_ids=list(range(N_CORES)),
                               trace=bool(os.environ.get("BASS_KERNEL_TRACE")))
    _COMPILED["last_result"] = res
    out = np.zeros((B, S, H), dtype=np.float32)
    for c in range(N_CORES):
        b = c // 4
        q0 = (c % 4) * I_CORE
        out[b, q0:q0 + I_CORE] = np.asarray(
            res.results[c]["out"]).reshape(I_CORE, H)
    return out
